# revision 9
# baseline (speedup 1.0000x reference)
"""Distributed Trainium2 kernel for batched multi-head self-attention with
positional bias.

Reference computation (per batch element b):
    qkv = x[b] @ w_qkv ; split into q,k,v ; heads of 64
    sim = (q * 64**-0.5) @ k^T + pos_bias          # [h, n, n]
    attn = softmax(sim, axis=-1)
    out[b] = (attn @ v).reshape(n, hidden) @ w_out

Sharding: pure data-parallel — core i computes batch element i (B == 8 ==
n_cores), no collectives.

Device algorithm (per core), designed to avoid all on-chip transposes:
  - host supplies xT = x[b].T, so projections produce Q^T,K^T ([d, n]) and V
    ([n, d]) directly with natural-layout matmuls.
  - attention is computed transposed: St[j,i] = sum_d K^T[d,j] Q^T[d,i];
    softmax over j is handled via exp (ScalarE) * exp(bias^T) (host
    precomputed, fp16) and a ones-block in the AV matmul's stationary
    operand, which makes PSUM rows 64:128 the softmax denominators.
  - U''[0:64] / U''[64:128] gives the normalized per-head context, already
    in the [hidden, n] layout the output projection needs as lhsT.
"""

import numpy as np

B, N, D = 8, 1024, 512
H, DH = 8, 64
SCALE = DH**-0.5
NCORES = 8
KT = D // 128  # 4 k-tiles over model dim / hidden dim
NJT = N // 128  # 8 j-tiles
IB = 512
NIB = N // IB  # 2 i-blocks

_CACHE = {}


def _build_graph(sim=False):
    import concourse.bass as bass
    import concourse.mybir as mybir
    from concourse import tile

    f32 = mybir.dt.float32
    f32r = mybir.dt.float32r
    f16 = mybir.dt.float16
    Exp = mybir.ActivationFunctionType.Exp

    import concourse.bacc as bacc

    # target_bir_lowering=False: bass/bacc lower to per-engine streams with
    # standalone waits itself; walrus's sync structs hold few waits and
    # reject Tile-generated multi-wait instructions otherwise.
    nc = bacc.Bacc(None, target_bir_lowering=False, debug=False)
    xT = nc.declare_dram_parameter("xT", [D, N], f16, isOutput=False)
    wqkv = nc.declare_dram_parameter("wqkv", [D, 3 * D], f16, isOutput=False)
    wout = nc.declare_dram_parameter("wout", [D, D], f16, isOutput=False)
    expbt = nc.declare_dram_parameter("expbt", [H, N, N], f16, isOutput=False)
    out = nc.declare_dram_parameter("out", [N, D], f32, isOutput=True)

    with tile.TileContext(nc) as tc:
        with (
            tc.tile_pool(name="const", bufs=1) as cpool,
            tc.tile_pool(name="mm_ps", bufs=2, space="PSUM") as mm_ps,
            tc.tile_pool(name="st_ps", bufs=2, space="PSUM") as st_ps,
            tc.tile_pool(name="u_ps", bufs=2, space="PSUM") as u_ps,
            tc.tile_pool(name="stream", bufs=3) as stream,
            tc.tile_pool(name="osb", bufs=2) as opool,
        ):
            # ---- Phase 0: resident loads ----
            w_sb = []
            xT_sb = []
            wout_sb = []
            for k in range(KT):
                w = cpool.tile([128, 3 * D], f16, tag=f"w{k}", name=f"w{k}")
                nc.sync.dma_start(w[:], wqkv[k * 128 : (k + 1) * 128, :])
                w_sb.append(w)
                xt = cpool.tile([128, N], f16, tag=f"xt{k}", name=f"xt{k}")
                nc.sync.dma_start(xt[:], xT[k * 128 : (k + 1) * 128, :])
                xT_sb.append(xt)
                wo = cpool.tile([128, D], f16, tag=f"wo{k}", name=f"wo{k}")
                nc.sync.dma_start(wo[:], wout[k * 128 : (k + 1) * 128, :])
                wout_sb.append(wo)

            # V1: per jt a [128, H*128] tensor holding, per head, the AV
            # stationary operand [v_h | ones] (ones via memset, v written by
            # the V projection). Per-jt tiles keep writer fan-in low so
            # matmul weight-loads don't exceed the sync-wait limit.
            V1_sb = []
            for jt in range(NJT):
                v1 = cpool.tile([128, H * 128], f16, tag=f"v1_{jt}", name=f"v1_{jt}")
                nc.vector.memset(v1[:], 1.0)
                V1_sb.append(v1)

            QT_sb = [cpool.tile([128, N], f16, tag=f"qt{t}", name=f"qt{t}") for t in range(KT)]
            KT_sb = [cpool.tile([128, N], f16, tag=f"kt{t}", name=f"kt{t}") for t in range(KT)]
            # Utn per (t, ib) to bound writer fan-in per tile at 2.
            Utn_sb = [
                [
                    cpool.tile([128, IB], f16, tag=f"ut{t}_{ib}", name=f"ut{t}_{ib}")
                    for ib in range(NIB)
                ]
                for t in range(KT)
            ]

            # ---- Phase 1: projections ----
            for t in range(KT):
                for ib in range(NIB):
                    for dst, col0 in ((QT_sb, 0), (KT_sb, D)):
                        ps = mm_ps.tile([128, IB], f32)
                        for k in range(KT):
                            nc.tensor.matmul(
                                ps[:],
                                w_sb[k][:, col0 + t * 128 : col0 + (t + 1) * 128],
                                xT_sb[k][:, ib * IB : (ib + 1) * IB],
                                start=(k == 0),
                                stop=(k == KT - 1),
                            )
                        nc.vector.tensor_copy(
                            dst[t][:, ib * IB : (ib + 1) * IB], ps[:]
                        )
            for nt in range(NJT):
                ps = mm_ps.tile([128, D], f32)
                for k in range(KT):
                    nc.tensor.matmul(
                        ps[:],
                        xT_sb[k][:, nt * 128 : (nt + 1) * 128],
                        w_sb[k][:, 2 * D : 3 * D],
                        start=(k == 0),
                        stop=(k == KT - 1),
                    )
                # scatter per-head 64-col slices into V1 cols 64:128
                # (cols 0:64 stay 1.0 so U'' rows 0:64 are the row-sums,
                # at base partition 0 where reciprocal_approx_fast works)
                nc.vector.tensor_copy(
                    V1_sb[nt].rearrange("p (h c) -> p h c", h=H)[:, :, DH : 2 * DH],
                    ps.rearrange("p (h c) -> p h c", h=H)[:, :, :],
                )

            # ---- Phase 2: attention (transposed) ----
            for h in range(H):
                t, po = h // 2, 64 * (h % 2)
                ebt_h = expbt[h].rearrange("(g two p) i -> g p two i", two=2, p=128)
                for ib in range(NIB):
                    ups = u_ps.tile([128, IB], f32)
                    for g in range(NJT // 2):
                        st = st_ps.tile([128, 2 * IB], f32)
                        for jj in range(2):
                            jt = 2 * g + jj
                            nc.tensor.matmul(
                                st[:, jj * IB : (jj + 1) * IB],
                                KT_sb[t][po : po + 64, jt * 128 : (jt + 1) * 128],
                                QT_sb[t][po : po + 64, ib * IB : (ib + 1) * IB],
                                start=True,
                                stop=True,
                            )
                        eb = stream.tile([128, 2 * IB], f16, tag="eb")
                        nc.sync.dma_start(
                            eb.rearrange("p (two i) -> p two i", two=2),
                            ebt_h[g, :, :, ib * IB : (ib + 1) * IB],
                        )
                        et0 = stream.tile([128, 2 * IB], f16, tag="et0")
                        nc.scalar.activation(et0[:], st[:], Exp)
                        et = stream.tile([128, 2 * IB], f16, tag="et")
                        nc.vector.tensor_mul(et[:], et0[:], eb[:])
                        for jj in range(2):
                            jt = 2 * g + jj
                            nc.tensor.matmul(
                                ups[:],
                                V1_sb[jt][:, h * 128 : (h + 1) * 128],
                                et[:, jj * IB : (jj + 1) * IB],
                                start=(jt == 0),
                                stop=(jt == NJT - 1),
                            )
                    rb = stream.tile([64, IB], f32, tag="rb")
                    nc.vector.reciprocal_approx_fast(rb[:, :], ups[0:64, :])
                    nc.vector.tensor_mul(
                        Utn_sb[t][ib][po : po + 64, :],
                        ups[64:128, :],
                        rb[:, :],
                    )

            # ---- Phase 3: output projection ----
            for nt in range(NJT):
                ps = mm_ps.tile([128, D], f32)
                for k in range(KT):
                    nc.tensor.matmul(
                        ps[:],
                        Utn_sb[k][nt // 4][:, (nt % 4) * 128 : (nt % 4 + 1) * 128],
                        wout_sb[k][:],
                        start=(k == 0),
                        stop=(k == KT - 1),
                    )
                osb = opool.tile([128, D], f32)
                nc.vector.tensor_copy(osb[:], ps[:])
                nc.sync.dma_start(out[nt * 128 : (nt + 1) * 128, :], osb[:])

    return nc


def _get_graph():
    if "nc" not in _CACHE:
        nc = _build_graph()
        nc.compile()
        _CACHE["nc"] = nc
    return _CACHE["nc"]


def _prep_inputs(x, pos_bias, w_qkv, w_out):
    x = np.asarray(x, dtype=np.float32)
    pos_bias = np.asarray(pos_bias, dtype=np.float32)
    w_qkv = np.asarray(w_qkv, dtype=np.float32)
    w_out = np.asarray(w_out, dtype=np.float32)

    wqkv_mod = w_qkv.copy()
    wqkv_mod[:, :D] *= SCALE
    wout16 = w_out.astype(np.float16)
    wqkv16 = wqkv_mod.astype(np.float16)
    expbt = np.exp(pos_bias.transpose(0, 2, 1)).astype(np.float16)
    expbt = np.ascontiguousarray(expbt)

    in_maps = []
    for b in range(NCORES):
        in_maps.append(
            {
                "xT": np.ascontiguousarray(x[b].T.astype(np.float16)),
                "wqkv": wqkv16,
                "wout": wout16,
                "expbt": expbt,
            }
        )
    return in_maps


def _run(x, pos_bias, w_qkv, w_out, trace=False):
    from concourse.bass_utils import run_bass_kernel_spmd

    nc = _get_graph()
    in_maps = _prep_inputs(x, pos_bias, w_qkv, w_out)
    res = run_bass_kernel_spmd(
        nc, in_maps, core_ids=list(range(NCORES)), trace=trace
    )
    outs = np.stack([np.asarray(res.results[b]["out"]) for b in range(NCORES)])
    return outs.astype(np.float32), res


def kernel(x, pos_bias, w_qkv, w_out):
    outs, _ = _run(x, pos_bias, w_qkv, w_out, trace=False)
    return outs
